# revision 2
# baseline (speedup 1.0000x reference)
"""Trainium2 Bass kernel for nn_PluckerEncoder — v2 (pipelined).

Computation per batch element (L=4096, D=1024, d_red=32, delta=1):
  z = h @ W_red + b_red                                   (L, 32)
  p[t,(i,j)] = z[t,i]*z[t-d,j] - z[t,j]*z[t-d,i],  i<j    (496 pairs)
  g[t] = (p / max(||p||, 1e-8)) @ W_plu + b_plu           (t >= d; else 0)

Sharding: data-parallel over batch B=8 -> one batch element per core.

v2 structure (vs the phase-sequential baseline):
  * One fused per-512-token-block pipeline (load -> transpose -> z -> gather
    -> pairs -> norm -> main matmul -> store) so all engines overlap.
  * ||p||^2 via the Lagrange identity ||z_t||^2*||z_d||^2 - (z_t.z_d)^2:
    kills the gpsimd p^2 pass and the 4-chunk ones-matmul reduction.
  * Gather/s waves aligned to z production ([H+t0, H+t0+T)); the halo
    cols [0, H) are zero and handled by prologue memsets, so block b's
    tail needs nothing from block b+1.
  * Pair rows repacked: chunk 3 rows 16..127 hold pairs 384..495, row 0
    holds the norm (32-aligned DVE write; wplu row (3,0) holds b_plu so
    the main matmul accumulates p@W + norm*b_plu).
  * norm row gets sqrt(max(n2,0)) (0 for t<delta -> g=0 rows exact);
    the scale row is rsqrt(max(n2, 3e-15)) so r*norm = 1 on valid rows.
  * PSUM: gathers [128,4,512] (4 banks) + u rotation (2) + z/s/d/rp (2x1).

Token layout on device: features on partitions, tokens on the free dim.
z^T lives in a [128, L+delta] halo'd buffer (left halo zeroed).
"""

import sys

sys.path.insert(0, "/opt/trn_rl_repo")

import numpy as np
import ml_dtypes

import concourse.bass as bass
import concourse.mybir as mybir
import concourse.tile as tile
import concourse.bacc as bacc
from concourse import bass_utils

F32 = mybir.dt.float32
BF16 = mybir.dt.bfloat16

D_RED = 32
IDX_I, IDX_J = np.triu_indices(D_RED, k=1)
NPAIR = IDX_I.size  # 496
KC = 4              # pair chunks of 128 (496 pairs + norm row + 15 zero pads)


def _pair_slot(k):
    """(chunk, row) for pair k. Chunks 0-2 dense; chunk 3 rows 16..127 so
    that chunk-3 row 0 is free for the norm row (32-aligned partition)."""
    if k < 384:
        return k // 128, k % 128
    return 3, 16 + (k - 384)


def _selection_consts():
    """selP[32q + r, kind, m] = 1 iff idx_<kind>(pair_at(q, m)) == r."""
    S = np.zeros((128, 2, 128), np.float32)
    for k in range(NPAIR):
        q, m = _pair_slot(k)
        S[32 * q + IDX_I[k], 0, m] = 1.0
        S[32 * q + IDX_J[k], 1, m] = 1.0
    return S.astype(ml_dtypes.bfloat16)


def build_program(L, D, delta, n_cores=8, T=512, repeat=1):
    assert L % T == 0 and D == 1024
    H = delta
    assert 1 <= H <= 128
    NB = L // T
    LH = L + H
    nc = bacc.Bacc("TRN2", target_bir_lowering=False, debug=False,
                   num_devices=n_cores)

    h_in = nc.dram_tensor("h_bf16", [L, D], BF16, kind="ExternalInput")
    wred_in = nc.dram_tensor("wred_rep", [128, 8, 128], BF16, kind="ExternalInput")
    bred_in = nc.dram_tensor("bred_rep", [128, 1], F32, kind="ExternalInput")
    wplu_in = nc.dram_tensor("wplu_ext", [128, KC, D], BF16, kind="ExternalInput")
    g_out = nc.dram_tensor("g", [L, D], F32, kind="ExternalOutput")

    sel_c = nc.inline_tensor(_selection_consts(), name="sel_const")
    onescol_c = nc.inline_tensor(np.ones((128, 1), ml_dtypes.bfloat16), name="onescol")
    one_c = nc.inline_tensor(np.ones((1, 1), np.float32), name="one_const")

    with tile.TileContext(nc) as tc:
        with (
            tc.tile_pool(name="persist", bufs=1) as persist,
            tc.tile_pool(name="hstage", bufs=3) as hstage,
            tc.tile_pool(name="hT", bufs=3) as hTp,
            tc.tile_pool(name="work", bufs=3) as work,
            tc.tile_pool(name="rows", bufs=2) as rows,
            tc.tile_pool(name="gout", bufs=6) as goutp,
            tc.tile_pool(name="gwin", bufs=3) as gwin,
            tc.tile_pool(name="pg", bufs=1, space="PSUM") as pg,
            tc.tile_pool(name="pu", bufs=3, space="PSUM") as pu,
            tc.tile_pool(name="pz", bufs=1, space="PSUM") as pz,
        ):
            # ---- tiles ----
            wrep = persist.tile([128, 8, 128], BF16)
            bred = persist.tile([128, 1], F32)
            wplu = persist.tile([128, KC, D], BF16)
            sel = persist.tile([128, 2, 128], BF16)
            onescol = persist.tile([128, 1], BF16)
            one11 = persist.tile([1, 1], F32)
            zr = persist.tile([128, LH], BF16, padded_shape=[128, LH + 31])
            s_row = persist.tile([1, LH], F32, padded_shape=[1, LH + 31])

            def load_block(b):
                t0 = b * T
                h_nat = hstage.tile([128, 4, D], BF16, name="h_nat")
                nc.sync.dma_start(
                    out=h_nat[:],
                    in_=h_in.ap()[t0:t0 + T, :].rearrange(
                        "(a p) d -> p a d", p=128))
                hT = hTp.tile([128, 8, T], BF16, name="hT")
                for a in range(4):
                    nc.sync.dma_start_transpose(
                        out=hT[:, :, a * 128:(a + 1) * 128],
                        in_=h_nat[:, a, :])
                return hT

            # block-0 load first, then weights (shortens pipeline fill)
            hT0 = load_block(0)
            nc.sync.dma_start(wrep[:], wred_in.ap())
            nc.sync.dma_start(bred[:], bred_in.ap())
            nc.sync.dma_start(sel[:], sel_c.ap())
            nc.sync.dma_start(onescol[:], onescol_c.ap())
            nc.sync.dma_start(one11[:], one_c.ap())
            nc.sync.dma_start(wplu[:], wplu_in.ap())

            def pairs(b, nr, Gw, Gprev):
                """E/F muls + subs for block b reading the rotating gather
                window: t-side = Gw (wave b); d-side = Gw shifted by H with
                the first H cols from Gprev's tail (zeros for b == 0)."""
                p_all = work.tile([128, KC, T], BF16, name="p_all")
                if Gprev is None:
                    nc.gpsimd.memset(p_all[:, :, 0:H], 0.0)
                for q in range(KC):
                    E = work.tile([128, T], BF16, name="E")
                    F = work.tile([128, T], BF16, name="F")
                    nc.vector.tensor_mul(E[:, H:T], Gw[:, 0, q, H:T],
                                         Gw[:, 1, q, 0:T - H])
                    nc.vector.tensor_mul(F[:, H:T], Gw[:, 1, q, H:T],
                                         Gw[:, 0, q, 0:T - H])
                    nc.gpsimd.tensor_sub(p_all[:, q, H:T], E[:, H:T], F[:, H:T])
                    if Gprev is not None:
                        nc.gpsimd.tensor_mul(E[:, 0:H], Gw[:, 0, q, 0:H],
                                             Gprev[:, 1, q, T - H:T])
                        nc.gpsimd.tensor_mul(F[:, 0:H], Gw[:, 1, q, 0:H],
                                             Gprev[:, 0, q, T - H:T])
                        nc.gpsimd.tensor_sub(p_all[:, q, 0:H], E[:, 0:H],
                                             F[:, 0:H])
                # norm into chunk-3 row 0 (aligned single-partition copy)
                nc.scalar.copy(p_all[0:1, 3, :], nr[:])
                return p_all

            def main_mm(b, p_all, rrow):
                """Scale columns, main matmul, scaled evacuation, store."""
                t0 = b * T
                rp4 = pz.tile([128, 4], F32, name="rp4", tag="zsd")
                for m in range(4):
                    nc.tensor.matmul(rp4[:, m:m + 1],
                                     rrow[:, m * 128:(m + 1) * 128],
                                     one11[:], start=True, stop=True)
                rcol4 = work.tile([128, 4], F32, name="rcol4")
                nc.vector.tensor_copy(rcol4[:], rp4[:])
                for m in range(T // 128):
                    c0 = m * 128
                    u0 = pu.tile([128, 512], F32, name="u0", tag="u")
                    u1 = pu.tile([128, 512], F32, name="u1", tag="u")
                    for q in range(KC):
                        nc.tensor.matmul(u0[:], p_all[:, q, c0:c0 + 128],
                                         wplu[:, q, 0:512],
                                         start=(q == 0), stop=(q == KC - 1))
                    for q in range(KC):
                        nc.tensor.matmul(u1[:], p_all[:, q, c0:c0 + 128],
                                         wplu[:, q, 512:1024],
                                         start=(q == 0), stop=(q == KC - 1))
                    gt = goutp.tile([128, D], F32, name="gt")
                    nc.scalar.activation(gt[:, 0:512], u0[:],
                                         mybir.ActivationFunctionType.Copy,
                                         scale=rcol4[:, m:m + 1])
                    nc.vector.tensor_scalar_mul(gt[:, 512:1024], u1[:],
                                                rcol4[:, m:m + 1])
                    nc.sync.dma_start(g_out.ap()[t0 + c0:t0 + c0 + 128, :],
                                      gt[:])

            for _ in range(repeat):
                # halo cols [0, H): z = 0 -> gathers/s are 0 there too
                nc.vector.memset(zr[:, 0:H], 0.0)
                nc.vector.memset(s_row[0:1, 0:H], 0.0)

                prev = None  # (nr, rrow, Gw) of block b-1
                G_pp = None  # window of block b-2
                for b in range(NB):
                    t0 = b * T
                    hT = load_block(b) if b > 0 else hT0

                    # ---- z block b (4x replicated on partitions) ----
                    zp = pz.tile([128, T], F32, name="zp", tag="zsd")
                    for c in range(8):
                        nc.tensor.matmul(zp[:], wrep[:, c, :], hT[:, c, :],
                                         start=(c == 0), stop=(c == 7))
                    nc.vector.tensor_scalar_add(zr[:, H + t0:H + t0 + T], zp[:],
                                                bred[:])

                    # ---- gathers over zr cols [H+t0, H+t0+T) ----
                    Gw = gwin.tile([128, 2, KC, T], BF16, name="Gw")
                    for kind in range(2):
                        gp = pg.tile([128, KC, T], F32, name="gp", tag="gp")
                        for q in range(KC):
                            nc.tensor.matmul(
                                gp[:, q, :],
                                sel[32 * q:32 * q + 32, kind, :],
                                zr[32 * q:32 * q + 32, H + t0:H + t0 + T],
                                start=True, stop=True,
                                tile_position=(32 * q, 0))
                        nc.scalar.copy(Gw[:, kind, :, :], gp[:])

                    # ---- previous block: pairs (E/F start off gathers b-1) ----
                    p_prev = (pairs(b - 1, prev[0], prev[2], G_pp)
                              if b >= 1 else None)

                    # ---- Lagrange sums: s = ||z||^2, d = z_t.z_d ----
                    sq = work.tile([128, T], BF16, name="sq")
                    nc.vector.tensor_mul(sq[:], zr[:, H + t0:H + t0 + T],
                                         zr[:, H + t0:H + t0 + T])
                    sp = pz.tile([1, T], F32, name="sp", tag="zsd")
                    nc.tensor.matmul(sp[0:1, :], onescol[:], sq[:],
                                     start=True, stop=True)
                    # 4x replicated rows -> scale 1/4 on evacuation
                    nc.scalar.activation(s_row[0:1, H + t0:H + t0 + T],
                                         sp[0:1, :],
                                         mybir.ActivationFunctionType.Copy,
                                         scale=0.25)
                    prod = work.tile([128, T], BF16, name="prod")
                    nc.vector.tensor_mul(prod[:], zr[:, H + t0:H + t0 + T],
                                         zr[:, t0:t0 + T])
                    dp = pz.tile([1, T], F32, name="dp", tag="zsd")
                    nc.tensor.matmul(dp[0:1, :], onescol[:], prod[:],
                                     start=True, stop=True)
                    d_row = rows.tile([1, T], F32, name="d_row")
                    nc.vector.tensor_scalar_mul(d_row[:], dp[0:1, :], 0.25)

                    # ---- norm rows for block b (consumed by iter b+1) ----
                    st_sd = rows.tile([1, T], F32, name="st_sd")
                    nc.vector.tensor_mul(st_sd[:], s_row[0:1, H + t0:H + t0 + T],
                                         s_row[0:1, t0:t0 + T])
                    d2 = rows.tile([1, T], F32, name="d2")
                    nc.scalar.activation(d2[:], d_row[:],
                                         mybir.ActivationFunctionType.Square)
                    n2 = rows.tile([1, T], F32, name="n2")
                    nc.vector.tensor_sub(n2[:], st_sd[:], d2[:])
                    n2c0 = rows.tile([1, T], F32, name="n2c0")
                    nc.vector.tensor_scalar_max(n2c0[:], n2[:], 0.0)
                    nr = rows.tile([1, T], F32, name="nr")
                    nc.scalar.activation(nr[:], n2c0[:],
                                         mybir.ActivationFunctionType.Sqrt)
                    nr2 = rows.tile([1, T], F32, name="nr2")
                    nc.vector.tensor_scalar_max(nr2[:], nr[:], 1e-8)
                    rrow = rows.tile([1, T], F32, name="rrow")
                    nc.vector.reciprocal(rrow[:], nr2[:])

                    # ---- previous block: main matmul + store ----
                    if b >= 1:
                        main_mm(b - 1, p_prev, prev[1])
                        G_pp = prev[2]
                    prev = (nr, rrow, Gw)

                # drain: last block's tail
                p_prev = pairs(NB - 1, prev[0], prev[2], G_pp)
                main_mm(NB - 1, p_prev, prev[1])
    nc.compile()
    return nc


_WEIGHT_CACHE = {}


def _weight_inputs(W_red_w, W_red_b, W_plu_w, W_plu_b, D):
    """Shared (replicated) weight tensors, cached across calls."""
    key = (W_red_w.ctypes.data, W_plu_w.ctypes.data, W_red_w.shape,
           float(W_red_w.flat[0]), float(W_plu_w.flat[0]),
           float(W_red_b.flat[0]), float(W_plu_b.flat[0]))
    hit = _WEIGHT_CACHE.get(key)
    if hit is not None:
        return hit
    bf = ml_dtypes.bfloat16
    wrep = np.ascontiguousarray(
        np.tile(W_red_w.reshape(8, 128, D_RED), (1, 1, 4)).transpose(1, 0, 2)
    ).astype(bf)  # [128, 8, 128]
    wplu_ext = np.zeros((KC * 128, D), np.float32)
    for k in range(NPAIR):
        q, m = _pair_slot(k)
        wplu_ext[q * 128 + m] = W_plu_w[k]
    wplu_ext[3 * 128 + 0] = W_plu_b  # norm row -> bias
    wplu = np.ascontiguousarray(
        wplu_ext.reshape(KC, 128, D).transpose(1, 0, 2)).astype(bf)
    bred = np.ascontiguousarray(np.tile(W_red_b, 4)[:, None]).astype(np.float32)
    out = {"wred_rep": wrep, "bred_rep": bred, "wplu_ext": wplu}
    _WEIGHT_CACHE.clear()
    _WEIGHT_CACHE[key] = out
    return out


def _host_inputs(h_b, W_red_w, W_red_b, W_plu_w, W_plu_b, D):
    """Per-core input dict (h_b is one batch element [L, D] f32)."""
    w = _weight_inputs(W_red_w, W_red_b, W_plu_w, W_plu_b, D)
    return {"h_bf16": h_b.astype(ml_dtypes.bfloat16), **w}


_PROGRAM_CACHE = {}


def _get_program(L, D, delta, n_cores, repeat=1):
    key = (L, D, delta, n_cores, repeat)
    if key not in _PROGRAM_CACHE:
        _PROGRAM_CACHE[key] = build_program(L, D, delta, n_cores=n_cores,
                                            repeat=repeat)
    return _PROGRAM_CACHE[key]


def kernel(h, window_offset, W_red_w, W_red_b, W_plu_w, W_plu_b, _repeat=1,
           _want_results=True, _phases=None):
    h = np.asarray(h)
    B, L, D = h.shape
    delta = int(window_offset)
    if delta >= L:
        return np.zeros_like(h, dtype=np.float32)
    nc = _get_program(L, D, delta, B, repeat=_repeat)
    in_maps = [
        _host_inputs(h[b], np.asarray(W_red_w), np.asarray(W_red_b),
                     np.asarray(W_plu_w), np.asarray(W_plu_b), D)
        for b in range(B)
    ]
    res = bass_utils.run_bass_kernel_spmd(nc, in_maps, core_ids=list(range(B)))
    if not _want_results:
        return None
    return np.stack([res.results[b]["g"] for b in range(B)], axis=0)


# revision 8
# speedup vs baseline: 6427.7295x; 6427.7295x over previous
"""Trainium2 Bass kernel for nn_PluckerEncoder — v2 (pipelined).

Computation per batch element (L=4096, D=1024, d_red=32, delta=1):
  z = h @ W_red + b_red                                   (L, 32)
  p[t,(i,j)] = z[t,i]*z[t-d,j] - z[t,j]*z[t-d,i],  i<j    (496 pairs)
  g[t] = (p / max(||p||, 1e-8)) @ W_plu + b_plu           (t >= d; else 0)

Sharding: data-parallel over batch B=8 -> one batch element per core.

v2 structure (vs the phase-sequential baseline):
  * One fused per-512-token-block pipeline (load -> transpose -> z -> gather
    -> pairs -> norm -> main matmul -> store) so all engines overlap.
  * ||p||^2 via the Lagrange identity ||z_t||^2*||z_d||^2 - (z_t.z_d)^2:
    kills the gpsimd p^2 pass and the 4-chunk ones-matmul reduction.
  * Gather/s waves aligned to z production ([H+t0, H+t0+T)); the halo
    cols [0, H) are zero and handled by prologue memsets, so block b's
    tail needs nothing from block b+1.
  * Pair rows repacked: chunk 3 rows 16..127 hold pairs 384..495, row 0
    holds the norm (32-aligned DVE write; wplu row (3,0) holds b_plu so
    the main matmul accumulates p@W + norm*b_plu).
  * norm row gets sqrt(max(n2,0)) (0 for t<delta -> g=0 rows exact);
    the scale row is rsqrt(max(n2, 3e-15)) so r*norm = 1 on valid rows.
  * PSUM: gathers [128,4,512] (4 banks) + u rotation (2) + z/s/d/rp (2x1).

Token layout on device: features on partitions, tokens on the free dim.
z^T lives in a [128, L+delta] halo'd buffer (left halo zeroed).
"""

import sys

sys.path.insert(0, "/opt/trn_rl_repo")

import numpy as np
import ml_dtypes

import concourse.bass as bass
import concourse.mybir as mybir
import concourse.tile as tile
import concourse.bacc as bacc
from concourse import bass_utils
from concourse._compat import axon_active

F32 = mybir.dt.float32
BF16 = mybir.dt.bfloat16

D_RED = 32
IDX_I, IDX_J = np.triu_indices(D_RED, k=1)
NPAIR = IDX_I.size  # 496
KC = 4              # pair chunks of 128 (496 pairs + norm row + 15 zero pads)


def _pair_slot(k):
    """(chunk, row) for pair k. Chunks 0-2 dense; chunk 3 rows 16..127 so
    that chunk-3 row 0 is free for the norm row (32-aligned partition)."""
    if k < 384:
        return k // 128, k % 128
    return 3, 16 + (k - 384)


def _selection_consts():
    """selP[32q + r, kind, m] = 1 iff idx_<kind>(pair_at(q, m)) == r."""
    S = np.zeros((128, 2, 128), np.float32)
    for k in range(NPAIR):
        q, m = _pair_slot(k)
        S[32 * q + IDX_I[k], 0, m] = 1.0
        S[32 * q + IDX_J[k], 1, m] = 1.0
    return S.astype(ml_dtypes.bfloat16)


def build_program(L, D, delta, n_cores=8, T=512, repeat=1):
    assert L % T == 0 and D == 1024
    H = delta
    assert 1 <= H <= 128
    NB = L // T
    LH = L + H
    nc = bacc.Bacc("TRN2", target_bir_lowering=False, debug=False,
                   num_devices=n_cores)

    h_in = nc.dram_tensor("h_bf16", [L, D], BF16, kind="ExternalInput")
    wred_in = nc.dram_tensor("wred_rep", [128, 8, 128], BF16, kind="ExternalInput")
    bred_in = nc.dram_tensor("bred_rep", [128, 1], F32, kind="ExternalInput")
    wplu_in = nc.dram_tensor("wplu_ext", [128, KC, D], BF16, kind="ExternalInput")
    g_out = nc.dram_tensor("g", [L, D], F32, kind="ExternalOutput")

    sel_c = nc.inline_tensor(_selection_consts(), name="sel_const")
    onescol_c = nc.inline_tensor(np.ones((128, 1), ml_dtypes.bfloat16), name="onescol")
    one_c = nc.inline_tensor(np.ones((1, 1), np.float32), name="one_const")

    with tile.TileContext(nc) as tc:
        with (
            tc.tile_pool(name="persist", bufs=1) as persist,
            tc.tile_pool(name="hstage", bufs=3) as hstage,
            tc.tile_pool(name="hT", bufs=3) as hTp,
            tc.tile_pool(name="work", bufs=3) as work,
            tc.tile_pool(name="rows", bufs=2) as rows,
            tc.tile_pool(name="gout", bufs=6) as goutp,
            tc.tile_pool(name="gwin", bufs=3) as gwin,
            tc.tile_pool(name="pg", bufs=1, space="PSUM") as pg,
            tc.tile_pool(name="pu", bufs=3, space="PSUM") as pu,
            tc.tile_pool(name="pz", bufs=1, space="PSUM") as pz,
        ):
            # ---- tiles ----
            wrep = persist.tile([128, 8, 128], BF16)
            bred = persist.tile([128, 1], F32)
            wplu = persist.tile([128, KC, D], BF16)
            sel = persist.tile([128, 2, 128], BF16)
            onescol = persist.tile([128, 1], BF16)
            one11 = persist.tile([1, 1], F32)
            zr = persist.tile([128, LH], BF16, padded_shape=[128, LH + 31])
            s_row = persist.tile([1, LH], F32, padded_shape=[1, LH + 31])

            def load_block(b):
                t0 = b * T
                h_nat = hstage.tile([128, 4, D], BF16, name="h_nat")
                nc.sync.dma_start(
                    out=h_nat[:],
                    in_=h_in.ap()[t0:t0 + T, :].rearrange(
                        "(a p) d -> p a d", p=128))
                hT = hTp.tile([128, 8, T], BF16, name="hT")
                for a in range(4):
                    nc.sync.dma_start_transpose(
                        out=hT[:, :, a * 128:(a + 1) * 128],
                        in_=h_nat[:, a, :])
                return hT

            # block-0 load first, then weights (shortens pipeline fill)
            hT0 = load_block(0)
            nc.sync.dma_start(wrep[:], wred_in.ap())
            nc.sync.dma_start(bred[:], bred_in.ap())
            nc.sync.dma_start(sel[:], sel_c.ap())
            nc.sync.dma_start(onescol[:], onescol_c.ap())
            nc.sync.dma_start(one11[:], one_c.ap())
            nc.sync.dma_start(wplu[:], wplu_in.ap())

            def pairs(b, nr, Gw, Gprev):
                """E/F muls + subs for block b reading the rotating gather
                window: t-side = Gw (wave b); d-side = Gw shifted by H with
                the first H cols from Gprev's tail (zeros for b == 0)."""
                p_all = work.tile([128, KC, T], BF16, name="p_all")
                if Gprev is None:
                    nc.gpsimd.memset(p_all[:, :, 0:H], 0.0)
                for q in range(KC):
                    E = work.tile([128, T], BF16, name="E")
                    F = work.tile([128, T], BF16, name="F")
                    nc.vector.tensor_mul(E[:, H:T], Gw[:, 0, q, H:T],
                                         Gw[:, 1, q, 0:T - H])
                    nc.vector.tensor_mul(F[:, H:T], Gw[:, 1, q, H:T],
                                         Gw[:, 0, q, 0:T - H])
                    nc.gpsimd.tensor_sub(p_all[:, q, H:T], E[:, H:T], F[:, H:T])
                    if Gprev is not None:
                        nc.gpsimd.tensor_mul(E[:, 0:H], Gw[:, 0, q, 0:H],
                                             Gprev[:, 1, q, T - H:T])
                        nc.gpsimd.tensor_mul(F[:, 0:H], Gw[:, 1, q, 0:H],
                                             Gprev[:, 0, q, T - H:T])
                        nc.gpsimd.tensor_sub(p_all[:, q, 0:H], E[:, 0:H],
                                             F[:, 0:H])
                # norm into chunk-3 row 0 (aligned single-partition copy)
                nc.scalar.copy(p_all[0:1, 3, :], nr[:])
                return p_all

            def main_mm(b, p_all, rrow):
                """Scale columns, main matmul, scaled evacuation, store."""
                t0 = b * T
                rp4 = pz.tile([128, 4], F32, name="rp4", tag="zsd")
                for m in range(4):
                    nc.tensor.matmul(rp4[:, m:m + 1],
                                     rrow[:, m * 128:(m + 1) * 128],
                                     one11[:], start=True, stop=True)
                rcol4 = work.tile([128, 4], F32, name="rcol4")
                nc.vector.tensor_copy(rcol4[:], rp4[:])
                for m in range(T // 128):
                    c0 = m * 128
                    u0 = pu.tile([128, 512], F32, name="u0", tag="u")
                    u1 = pu.tile([128, 512], F32, name="u1", tag="u")
                    for q in range(KC):
                        nc.tensor.matmul(u0[:], p_all[:, q, c0:c0 + 128],
                                         wplu[:, q, 0:512],
                                         start=(q == 0), stop=(q == KC - 1))
                    for q in range(KC):
                        nc.tensor.matmul(u1[:], p_all[:, q, c0:c0 + 128],
                                         wplu[:, q, 512:1024],
                                         start=(q == 0), stop=(q == KC - 1))
                    gt = goutp.tile([128, D], F32, name="gt")
                    nc.scalar.activation(gt[:, 0:512], u0[:],
                                         mybir.ActivationFunctionType.Copy,
                                         scale=rcol4[:, m:m + 1])
                    nc.vector.tensor_scalar_mul(gt[:, 512:1024], u1[:],
                                                rcol4[:, m:m + 1])
                    nc.sync.dma_start(g_out.ap()[t0 + c0:t0 + c0 + 128, :],
                                      gt[:])

            for _ in range(repeat):
                # halo cols [0, H): z = 0 -> gathers/s are 0 there too
                nc.vector.memset(zr[:, 0:H], 0.0)
                nc.vector.memset(s_row[0:1, 0:H], 0.0)

                prev = None  # (nr, rrow, Gw) of block b-1
                G_pp = None  # window of block b-2
                for b in range(NB):
                    t0 = b * T
                    hT = load_block(b) if b > 0 else hT0

                    # ---- z block b (4x replicated on partitions) ----
                    zp = pz.tile([128, T], F32, name="zp", tag="zsd")
                    for c in range(8):
                        nc.tensor.matmul(zp[:], wrep[:, c, :], hT[:, c, :],
                                         start=(c == 0), stop=(c == 7))
                    nc.vector.tensor_scalar_add(zr[:, H + t0:H + t0 + T], zp[:],
                                                bred[:])

                    # ---- gathers over zr cols [H+t0, H+t0+T) ----
                    Gw = gwin.tile([128, 2, KC, T], BF16, name="Gw")
                    for kind in range(2):
                        gp = pg.tile([128, KC, T], F32, name="gp", tag="gp")
                        for q in range(KC):
                            nc.tensor.matmul(
                                gp[:, q, :],
                                sel[32 * q:32 * q + 32, kind, :],
                                zr[32 * q:32 * q + 32, H + t0:H + t0 + T],
                                start=True, stop=True,
                                tile_position=(32 * q, 0))
                        nc.scalar.copy(Gw[:, kind, :, :], gp[:])

                    # ---- previous block: pairs (E/F start off gathers b-1) ----
                    p_prev = (pairs(b - 1, prev[0], prev[2], G_pp)
                              if b >= 1 else None)

                    # ---- Lagrange sums: s = ||z||^2, d = z_t.z_d ----
                    sq = work.tile([128, T], BF16, name="sq")
                    nc.vector.tensor_mul(sq[:], zr[:, H + t0:H + t0 + T],
                                         zr[:, H + t0:H + t0 + T])
                    sp = pz.tile([1, T], F32, name="sp", tag="zsd")
                    nc.tensor.matmul(sp[0:1, :], onescol[:], sq[:],
                                     start=True, stop=True)
                    # 4x replicated rows -> scale 1/4 on evacuation
                    nc.scalar.activation(s_row[0:1, H + t0:H + t0 + T],
                                         sp[0:1, :],
                                         mybir.ActivationFunctionType.Copy,
                                         scale=0.25)
                    prod = work.tile([128, T], BF16, name="prod")
                    nc.vector.tensor_mul(prod[:], zr[:, H + t0:H + t0 + T],
                                         zr[:, t0:t0 + T])
                    dp = pz.tile([1, T], F32, name="dp", tag="zsd")
                    nc.tensor.matmul(dp[0:1, :], onescol[:], prod[:],
                                     start=True, stop=True)
                    d_row = rows.tile([1, T], F32, name="d_row")
                    nc.vector.tensor_scalar_mul(d_row[:], dp[0:1, :], 0.25)

                    # ---- norm rows for block b (consumed by iter b+1) ----
                    st_sd = rows.tile([1, T], F32, name="st_sd")
                    nc.vector.tensor_mul(st_sd[:], s_row[0:1, H + t0:H + t0 + T],
                                         s_row[0:1, t0:t0 + T])
                    d2 = rows.tile([1, T], F32, name="d2")
                    nc.scalar.activation(d2[:], d_row[:],
                                         mybir.ActivationFunctionType.Square)
                    n2 = rows.tile([1, T], F32, name="n2")
                    nc.vector.tensor_sub(n2[:], st_sd[:], d2[:])
                    n2c0 = rows.tile([1, T], F32, name="n2c0")
                    nc.vector.tensor_scalar_max(n2c0[:], n2[:], 0.0)
                    nr = rows.tile([1, T], F32, name="nr")
                    nc.scalar.activation(nr[:], n2c0[:],
                                         mybir.ActivationFunctionType.Sqrt)
                    nr2 = rows.tile([1, T], F32, name="nr2")
                    nc.vector.tensor_scalar_max(nr2[:], nr[:], 1e-8)
                    rrow = rows.tile([1, T], F32, name="rrow")
                    nc.vector.reciprocal(rrow[:], nr2[:])

                    # ---- previous block: main matmul + store ----
                    if b >= 1:
                        main_mm(b - 1, p_prev, prev[1])
                        G_pp = prev[2]
                    prev = (nr, rrow, Gw)

                # drain: last block's tail
                p_prev = pairs(NB - 1, prev[0], prev[2], G_pp)
                main_mm(NB - 1, p_prev, prev[1])
    nc.compile()
    return nc


_WEIGHT_CACHE = {}


def _weight_inputs(W_red_w, W_red_b, W_plu_w, W_plu_b, D):
    """Shared (replicated) weight tensors, cached across calls."""
    def _fp(a):
        f = np.ascontiguousarray(a).view(np.uint8).ravel()
        step = max(1, f.size // 1024)
        return (a.shape, str(a.dtype), f[::step].tobytes())
    key = (_fp(W_red_w), _fp(W_red_b), _fp(W_plu_w), _fp(W_plu_b))
    hit = _WEIGHT_CACHE.get(key)
    if hit is not None:
        return hit
    bf = ml_dtypes.bfloat16
    wrep = np.ascontiguousarray(
        np.tile(W_red_w.reshape(8, 128, D_RED), (1, 1, 4)).transpose(1, 0, 2)
    ).astype(bf)  # [128, 8, 128]
    wplu_ext = np.zeros((KC * 128, D), np.float32)
    for k in range(NPAIR):
        q, m = _pair_slot(k)
        wplu_ext[q * 128 + m] = W_plu_w[k]
    wplu_ext[3 * 128 + 0] = W_plu_b  # norm row -> bias
    wplu = np.ascontiguousarray(
        wplu_ext.reshape(KC, 128, D).transpose(1, 0, 2)).astype(bf)
    bred = np.ascontiguousarray(np.tile(W_red_b, 4)[:, None]).astype(np.float32)
    out = {"wred_rep": wrep, "bred_rep": bred, "wplu_ext": wplu}
    _WEIGHT_CACHE.clear()
    _WEIGHT_CACHE[key] = out
    return out


def _host_inputs(h_b, W_red_w, W_red_b, W_plu_w, W_plu_b, D):
    """Per-core input dict (h_b is one batch element [L, D] f32)."""
    w = _weight_inputs(W_red_w, W_red_b, W_plu_w, W_plu_b, D)
    return {"h_bf16": h_b.astype(ml_dtypes.bfloat16), **w}


_PROGRAM_CACHE = {}


def _get_program(L, D, delta, n_cores, repeat=1):
    key = (L, D, delta, n_cores, repeat)
    if key not in _PROGRAM_CACHE:
        _PROGRAM_CACHE[key] = build_program(L, D, delta, n_cores=n_cores,
                                            repeat=repeat)
    return _PROGRAM_CACHE[key]


class _AxonRunner:
    """Persistent executor for one compiled program under axon.

    run_bass_kernel_spmd -> bass2jax.run_bass_via_pjrt rebuilds its jitted
    shard_map closure on every call, which re-traces, re-lowers and re-loads
    the executable each time (seconds per call).  This runner builds the
    identical _bass_exec_p computation ONCE and keeps the jitted function,
    the device-resident replicated weights, and the donated-zero output
    seeds alive across calls, so a steady-state call only uploads h and
    downloads g.
    """

    def __init__(self, nc, n_cores):
        import jax
        from jax.sharding import Mesh, PartitionSpec, NamedSharding
        from jax.experimental.shard_map import shard_map
        from concourse.bass2jax import (_bass_exec_p, install_neuronx_cc_hook,
                                        partition_id_tensor)

        install_neuronx_cc_hook()
        self._jax = jax
        self.n_cores = n_cores
        partition_name = (nc.partition_id_tensor.name
                          if nc.partition_id_tensor else None)
        in_names, out_names, out_avals, zero_shapes = [], [], [], []
        for alloc in nc.m.functions[0].allocations:
            if not isinstance(alloc, mybir.MemoryLocationSet):
                continue
            name = alloc.memorylocations[0].name
            if alloc.kind == "ExternalInput":
                if name != partition_name:
                    in_names.append(name)
            elif alloc.kind == "ExternalOutput":
                out_names.append(name)
                shape = tuple(alloc.tensor_shape)
                dtype = mybir.dt.np(alloc.dtype)
                out_avals.append(jax.core.ShapedArray(shape, dtype))
                zero_shapes.append((shape, dtype))
        self.in_names = in_names
        self.out_names = out_names
        self.out_avals = out_avals
        all_in_names = list(in_names) + list(out_names)
        if partition_name is not None:
            all_in_names.append(partition_name)

        def _body(*args):
            operands = list(args)
            if partition_name is not None:
                operands.append(partition_id_tensor())
            outs = _bass_exec_p.bind(
                *operands,
                out_avals=tuple(out_avals),
                in_names=tuple(all_in_names),
                out_names=tuple(out_names),
                lowering_input_output_aliases=(),
                sim_require_finite=True,
                sim_require_nnan=True,
                nc=nc,
            )
            return tuple(outs)

        devices = jax.devices()[:n_cores]
        mesh = Mesh(np.asarray(devices), ("core",))
        n_in = len(in_names) + len(zero_shapes)
        self.fn = jax.jit(
            shard_map(_body, mesh=mesh,
                      in_specs=(PartitionSpec("core"),) * n_in,
                      out_specs=(PartitionSpec("core"),) * len(out_names),
                      check_rep=False),
            keep_unused=True,
        )
        self.sharding = NamedSharding(mesh, PartitionSpec("core"))
        self.dev_zeros = [
            jax.device_put(np.zeros((n_cores * s[0], *s[1:]), dt),
                           self.sharding)
            for s, dt in zero_shapes
        ]
        self._dev_cache = {}

    def run(self, in_maps):
        """in_maps: per-core dicts. Static tensors (everything except
        h_bf16) are device-cached keyed by a cheap fingerprint."""
        jax = self._jax
        args = []
        for name in self.in_names:
            per_core = [np.asarray(m[name]) for m in in_maps]
            if name == "h_bf16":
                arr = np.concatenate(per_core, axis=0)
                args.append(jax.device_put(arr, self.sharding))
            else:
                a0 = per_core[0]
                key = (name, a0.ctypes.data, a0.shape, a0.ravel()[:4].tobytes())
                hit = self._dev_cache.get(key)
                if hit is None:
                    arr = np.concatenate(per_core, axis=0)
                    hit = jax.device_put(arr, self.sharding)
                    self._dev_cache = {k: v for k, v in
                                       self._dev_cache.items() if k[0] != name}
                    self._dev_cache[key] = hit
                args.append(hit)
        outs = self.fn(*args, *self.dev_zeros)
        # one download per output; per-core entries are views of it
        full = {
            name: np.asarray(outs[i]).reshape(self.n_cores,
                                              *self.out_avals[i].shape)
            for i, name in enumerate(self.out_names)
        }
        res = [{name: full[name][c] for name in self.out_names}
               for c in range(self.n_cores)]
        return res, full


_RUNNER_CACHE = {}


def _run(nc, in_maps, n_cores, key):
    """Returns (per_core_results, full_outputs_or_None)."""
    if not axon_active():
        res = bass_utils.run_bass_kernel_spmd(nc, in_maps,
                                              core_ids=list(range(n_cores)))
        return res.results, None
    runner = _RUNNER_CACHE.get(key)
    if runner is None:
        runner = _AxonRunner(nc, n_cores)
        _RUNNER_CACHE.clear()
        _RUNNER_CACHE[key] = runner
    return runner.run(in_maps)


def kernel(h, window_offset, W_red_w, W_red_b, W_plu_w, W_plu_b, _repeat=1,
           _want_results=True, _phases=None):
    h = np.asarray(h)
    B, L, D = h.shape
    delta = int(window_offset)
    if delta >= L:
        return np.zeros_like(h, dtype=np.float32)
    nc = _get_program(L, D, delta, B, repeat=_repeat)
    in_maps = [
        _host_inputs(h[b], np.asarray(W_red_w), np.asarray(W_red_b),
                     np.asarray(W_plu_w), np.asarray(W_plu_b), D)
        for b in range(B)
    ]
    results, full = _run(nc, in_maps, B, key=(L, D, delta, B, _repeat))
    if not _want_results:
        return None
    if full is not None:
        return full["g"]  # already (B, L, D), no copy
    return np.stack([results[b]["g"] for b in range(B)], axis=0)
